# revision 39
# baseline (speedup 1.0000x reference)
"""Trainium2 Bass kernel for nn_CustomFullyConnectedLayerGoogleTopK.

Reference computation:
    a = clip(K * softmax(alpha), 0, 1)                    # (4096,)
    W[rows, cols] += (V * a[:, None])  with rows=(j+i)%N, cols=j
    out = x @ W.T                                          # (256, 4096)

The scatter indices form a bijection (for each col j, row (j+i)%N hits every
row exactly once as i varies), so there is no actual accumulation:

    W[r, c] = V[(r - c) % N, c] * a[(r - c) % N]
    out[b, r] = sum_c x[b, c] * V[(r-c)%N, c] * a[(r-c)%N]

Sharding: output columns r are sharded 8 ways (512 per core) -> no collective;
each core reads only the diagonal band of V it needs, all of x, and produces a
disjoint out[:, r0:r0+512] slice.

The GEMM datapath runs in bf16 (tolerance is 2e-2; measured error ~4e-3):
the host ships the V band, x^T and alpha already in bf16 and in
partition-blocked layouts so every DMA is 128 large contiguous descriptors
(2-8 KB each).  Per-core HBM traffic is ~8 MB (band 4 MB + xT 2 MB +
Toeplitz scale windows 1.4 MB + out 0.25 MB) vs 16.8 MB for the fp32
version, and sustains 420+ GB/s.

Device-side layout trick: with the contraction rows presented in REVERSED
order (c = N-1-p for SBUF partition-row p), the skewed scale field the band
tiles need becomes the ascending Toeplitz  scale[p, j] = a2[1 + p + j]  where
a2 is `a` doubled.  Raw (doubled, rolled) alpha is DMA'd directly in this
overlapping-window layout (one DMA per batch, partition step +1); the
soft-topk transform is applied on-chip:

    a = min(exp(alpha) * K/sum(exp(alpha)), 1)

via one Exp pass per window on the otherwise-idle Scalar engine and a fused
mult+min tensor_scalar on Vector (bf16 single-src -> 4x mode), with 1/sum
from a ones-matmul partition broadcast + DVE reciprocal.  Only the Exp
activation table is loaded (less preamble); nothing round-trips through
DRAM.  (GpSimd's tensor_scalar ucode for this shape measured 17x slower
than DVE -- do not move the scale pass there.)

DMA scheduling (load-bearing, learned from traces):
  * ALL loads ride ONE HWDGE ring (sync) in strict batch order
    [alpha, (win_q, band_q, xt_q) x 5]: single-queue FIFO makes completion
    order == need order.  A second ring gets starved (SDMA round-robins
    rings per-DESCRIPTOR, so small-descriptor transfers lose ~4x to big
    ones), and the 8 shared completion-semaphore lanes recycle round-robin
    ACROSS rings, stalling later issues on whichever ring reuses a lane.
  * Keep 2-3 mid-size transfers (0.25-1 MB, 2-8 KB descriptors) in flight:
    one packed 1.9 MB transfer per batch (12-15 KB descriptors) measured
    ~280 GB/s vs 420+ for this mix.
  * Batch sizes [4, 8, 8, 8, 4]: small first batch starts the matmul
    pipeline early, small last batch keeps the post-DMA tail short.  The
    last batch loads xT before its band so the band is the only tail gate.
The two output halves take independent engine paths (DVE cast + scalar-ring
store / ACT copy + sync-ring store) so they fully overlap; tiny keep-alive
matmuls gated on each band arrival stop the PE HAM clock-gate from
re-throttling between matmul bursts.
"""

import os
import sys

import numpy as np

for _p in ("/opt/trn_rl_repo", "/root/.axon_site/_ro/trn_rl_repo"):
    if os.path.isdir(_p) and _p not in sys.path:
        sys.path.append(_p)

import ml_dtypes

import concourse.bacc as bacc
import concourse.bass as bass
import concourse.mybir as mybir
import concourse.tile as tile
from concourse.bass_utils import run_bass_kernel_spmd

F32 = mybir.dt.float32
BF16 = mybir.dt.bfloat16
NP_BF16 = ml_dtypes.bfloat16

N = 4096          # IN_F == OUT_F == N_PERM == DIAG
B = 256           # batch
NCORES = 8
RW = N // NCORES  # 512 output columns per core
K_TOPK = 3687     # ceil(0.9 * 4096 * 4096 / 4096)
CB = 128          # contraction block (SBUF partition count)
NCB = N // CB     # 32 contraction blocks
# contraction blocks per DMA/multiply batch: smaller first batch so the first
# matmuls start early, tiny last batch so the post-DMA tail is short (the PE
# has slack mid-stream, so front-loading blocks there costs nothing)
BATCH_SIZES = [6, 8, 8, 8, 2]
BATCH_STARTS = [sum(BATCH_SIZES[:i]) for i in range(len(BATCH_SIZES))]
NBATCH = len(BATCH_SIZES)


def _strided_cols(ap2d, col_off, t_step, n_t, inner):
    """[128, W] SBUF tile -> [128, n_t, inner] view starting at col_off with
    column stride t_step between t-slices (overlap allowed)."""
    pstep = ap2d.ap[0][0]
    return bass.AP(
        ap2d.tensor, ap2d.offset + col_off,
        [[pstep, 128], [t_step, n_t], [1, inner]],
    )


def _build_program():
    nc = bacc.Bacc("TRN2", target_bir_lowering=False, debug=False)

    band = nc.dram_tensor("band", [128, NCB, RW], BF16, kind="ExternalInput").ap()
    xT = nc.dram_tensor("xT", [128, NCB, B], BF16, kind="ExternalInput").ap()
    alpha2 = nc.dram_tensor("alpha2", [2 * N], BF16, kind="ExternalInput").ap()
    out = nc.dram_tensor("out", [B, RW], BF16, kind="ExternalOutput").ap()

    with tile.TileContext(nc) as tc:
        with (
            tc.tile_pool(name="small", bufs=1) as sp,
            tc.tile_pool(name="graw", bufs=1) as grp,
            tc.tile_pool(name="gexp", bufs=3) as gxp,
            tc.tile_pool(name="gsc", bufs=3) as gwp,
            tc.tile_pool(name="vb", bufs=1) as vbp,
            tc.tile_pool(name="xtp", bufs=1) as xtp,
            tc.tile_pool(name="wt", bufs=4) as wtp,
            tc.tile_pool(name="opool", bufs=2) as op,
            tc.tile_pool(name="psum", bufs=1, space="PSUM") as pp,
            tc.tile_pool(name="psum_s", bufs=1, space="PSUM") as pps,
        ):
            # ---- input DMAs ----
            # ALL loads ride ONE HWDGE ring (sync) in strict batch order
            # [alpha, (w_q, band_q, xt_q) x 5]: single-queue FIFO makes
            # completion order == need order, and the mix of 2-8KB
            # descriptors across 2-3 in-flight transfers keeps the SDMA
            # engines fed (measured 420+ GB/s; a second ring gets starved
            # per-descriptor, and single huge packed transfers drop to
            # ~280 GB/s).
            alpha_sb = sp.tile([128, 2 * N // 128], BF16)
            nc.sync.dma_start(
                alpha_sb[:], alpha2[0 : 2 * N].rearrange("(p f) -> p f", p=128)
            )
            agr = [
                grp.tile([128, RW + (BATCH_SIZES[q] - 1) * CB], BF16,
                         name=f"agr{q}")
                for q in range(NBATCH)
            ]
            vb = [
                vbp.tile([128, BATCH_SIZES[q], RW], BF16, name=f"vb{q}")
                for q in range(NBATCH)
            ]
            xt = xtp.tile([128, NCB, B], BF16)

            def _dma_w(q):
                gpw = RW + (BATCH_SIZES[q] - 1) * CB
                src = bass.AP(
                    alpha2.tensor,
                    alpha2.offset + 1 + BATCH_STARTS[q] * CB,
                    [[1, 128], [1, gpw]],
                )
                nc.sync.dma_start(agr[q][:], src)

            def _dma_band(q):
                k0, tb = BATCH_STARTS[q], BATCH_SIZES[q]
                nc.sync.dma_start(vb[q][:], band[:, k0 : k0 + tb, :])

            def _dma_xt(q):
                k0, tb = BATCH_STARTS[q], BATCH_SIZES[q]
                nc.sync.dma_start(
                    xt[:, k0 : k0 + tb, :], xT[:, k0 : k0 + tb, :]
                )

            # steady batches stream [w_q, band_q, xt_q]; for the last batch
            # the xT chunk goes before the band, and the band arrives as two
            # half-DMAs, so the tail's TT/matmuls pipeline with the final
            # transfer instead of waiting for all of it
            for q in range(NBATCH - 1):
                _dma_w(q)
                _dma_band(q)
                _dma_xt(q)
            qL = NBATCH - 1
            k0L, tbL = BATCH_STARTS[qL], BATCH_SIZES[qL]
            hL = tbL // 2
            _dma_w(qL)
            _dma_xt(qL)
            nc.sync.dma_start(vb[qL][:, 0:hL, :], band[:, k0L : k0L + hL, :])
            nc.sync.dma_start(
                vb[qL][:, hL:tbL, :], band[:, k0L + hL : k0L + tbL, :]
            )

            def _vb(q):
                return vb[q][:]

            def _xt(q, t, b0, b1):
                return xt[:, BATCH_STARTS[q] + t, b0:b1]

            # ---- kinv = K / sum(exp(alpha)) broadcast to all partitions ----
            exp_sb = sp.tile([128, 2 * N // 128], F32)
            rowsum = sp.tile([128, 1], F32)
            # alpha is uniform in [0,1): no max-subtraction needed
            nc.scalar.activation(
                exp_sb[:], alpha_sb[:], mybir.ActivationFunctionType.Exp,
                accum_out=rowsum[:],
            )
            ones = sp.tile([128, 128], F32)
            nc.vector.memset(ones[:], 1.0)
            tot_ps = pps.tile([128, 1], F32)
            # total = ones.T @ rowsum -> per-partition copy of 2*sum
            nc.tensor.matmul(tot_ps[:], ones[:], rowsum[:], start=True, stop=True)
            inv = sp.tile([128, 1], F32)
            nc.vector.reciprocal(inv[:], tot_ps[:])
            kinv = sp.tile([128, 1], F32)
            # rowsum covered the doubled alpha -> tot = 2*sum, so scale by 2K
            nc.vector.tensor_scalar_mul(kinv[:], inv[:], 2.0 * K_TOPK)

            # ---- main loop ----
            psum0 = pp.tile([128, RW], F32)
            psum1 = pp.tile([128, RW], F32)
            psum_ka = pp.tile([128, 1], F32)
            for q in range(NBATCH):
                if q > 0:
                    # PE keep-alive: a tiny matmul gated on this batch's band
                    # arrival fires mid-gap between matmul bursts, so the
                    # HAM activity monitor never re-throttles the PE clock
                    # (cold matmuls run at 427-634ns instead of 216ns)
                    nc.tensor.matmul(
                        psum_ka[:], vb[q][:, 0, 0:128], vb[q][:, 0, 0:1],
                        start=True, stop=True,
                    )
                k0, tb = BATCH_STARTS[q], BATCH_SIZES[q]
                gpw = RW + (tb - 1) * CB
                # scale window: exp on Scalar (bf16 out), fused *kinv, min-1
                # on Vector (bf16 single-src -> 4x mode)
                agx = gxp.tile([128, gpw], BF16)
                nc.scalar.activation(
                    agx[:], agr[q][:], mybir.ActivationFunctionType.Exp
                )
                agw = gwp.tile([128, gpw], BF16)
                nc.vector.tensor_scalar(
                    agw[:], agx[:], kinv[:, 0:1], 1.0,
                    mybir.AluOpType.mult, mybir.AluOpType.min,
                )
                # scaled weights for this batch of tb contraction blocks
                wt = wtp.tile([128, tb, RW], BF16)
                if q < NBATCH - 1:
                    nc.vector.tensor_tensor(
                        wt[:], _vb(q), _strided_cols(agw, 0, CB, tb, RW),
                        mybir.AluOpType.mult,
                    )
                    for t in range(tb):
                        k = k0 + t
                        nc.tensor.matmul(psum0[:], _xt(q, t, 0, 128), wt[:, t, :],
                                         start=(k == 0), stop=False)
                        nc.tensor.matmul(psum1[:], _xt(q, t, 128, 256), wt[:, t, :],
                                         start=(k == 0), stop=False)
                else:
                    # tail: two half-TTs track the split band DMA; psum0's
                    # matmuls complete first so its cast+store overlaps
                    # psum1's remainder
                    h = tb // 2
                    nc.vector.tensor_tensor(
                        wt[:, 0:h, :], vb[q][:, 0:h, :],
                        _strided_cols(agw, 0, CB, h, RW),
                        mybir.AluOpType.mult,
                    )
                    nc.vector.tensor_tensor(
                        wt[:, h:tb, :], vb[q][:, h:tb, :],
                        _strided_cols(agw, h * CB, CB, tb - h, RW),
                        mybir.AluOpType.mult,
                    )
                    for t in range(h):
                        nc.tensor.matmul(psum0[:], _xt(q, t, 0, 128), wt[:, t, :],
                                         start=False, stop=False)
                        nc.tensor.matmul(psum1[:], _xt(q, t, 128, 256), wt[:, t, :],
                                         start=False, stop=False)
                    for t in range(h, tb):
                        k = k0 + t
                        nc.tensor.matmul(psum0[:], _xt(q, t, 0, 128), wt[:, t, :],
                                         start=False, stop=(k == NCB - 1))
                    for t in range(h, tb):
                        k = k0 + t
                        nc.tensor.matmul(psum1[:], _xt(q, t, 128, 256), wt[:, t, :],
                                         start=False, stop=(k == NCB - 1))

            # ---- PSUM -> SBUF -> DRAM (bf16 out; host widens to f32) ----
            # two independent engine paths so the halves fully overlap
            o0 = op.tile([128, RW], BF16)
            nc.vector.tensor_copy(o0[:], psum0[:])
            nc.scalar.dma_start(out[0:128, :], o0[:])
            o1 = op.tile([128, RW], BF16)
            nc.scalar.activation(
                o1[:], psum1[:], mybir.ActivationFunctionType.Copy
            )
            nc.sync.dma_start(out[128:256, :], o1[:])

    nc.compile()
    return nc


_NC_CACHE = []


def _get_program():
    if not _NC_CACHE:
        _NC_CACHE.append(_build_program())
    return _NC_CACHE[0]


def prepare_in_maps(x: np.ndarray, V: np.ndarray, alpha: np.ndarray):
    """Layout/dtype-only sharding of the full inputs into 8 per-core maps."""
    x = np.ascontiguousarray(np.asarray(x, dtype=np.float32))
    V = np.ascontiguousarray(np.asarray(V, dtype=np.float32))
    alpha = np.ascontiguousarray(np.asarray(alpha, dtype=np.float32))

    # rows presented in reversed order (c = N-1-p); see module docstring.
    # blocked [128, NCB, B] so each DMA chunk is contiguous per partition.
    xTb = np.ascontiguousarray(
        x.T[::-1, :].reshape(NCB, 128, B).transpose(1, 0, 2)
    ).astype(NP_BF16)

    # VtD[c, t] = V[t % N, c] for t in [0, 2N): doubled transpose for wrap-free
    # band extraction. band_m[c, j] = V[(r0 + j - c) % N, c]
    #              = VtD[c, N + r0 + j - c]
    Vt = np.ascontiguousarray(V.T)
    VtD = np.concatenate([Vt, Vt], axis=1)  # (N, 2N)
    flat = VtD.reshape(-1)
    isz = flat.itemsize

    in_maps = []
    for m in range(NCORES):
        r0 = m * RW
        start = N + r0  # element offset of band_m[0, 0] in flat
        band_m = np.lib.stride_tricks.as_strided(
            flat[start:], shape=(N, RW), strides=((2 * N - 1) * isz, isz),
        )
        band_b = np.ascontiguousarray(
            band_m[::-1, :].reshape(NCB, 128, RW).transpose(1, 0, 2)
        ).astype(NP_BF16)
        am = np.roll(alpha, -r0)
        in_maps.append({
            "band": band_b,
            "xT": xTb,
            "alpha2": np.concatenate([am, am]).astype(NP_BF16),
        })
    return in_maps


def gather_output(results) -> np.ndarray:
    return np.concatenate(
        [np.asarray(results[m]["out"], dtype=np.float32) for m in range(NCORES)],
        axis=1,
    )


def kernel(x: np.ndarray, V: np.ndarray, alpha: np.ndarray) -> np.ndarray:
    in_maps = prepare_in_maps(x, V, alpha)
    nc = _get_program()
    res = run_bass_kernel_spmd(nc, in_maps, core_ids=list(range(NCORES)))
    return gather_output(res.results)


# revision 40
# speedup vs baseline: 1.1660x; 1.1660x over previous
"""Trainium2 Bass kernel for nn_CustomFullyConnectedLayerGoogleTopK.

Reference computation:
    a = clip(K * softmax(alpha), 0, 1)                    # (4096,)
    W[rows, cols] += (V * a[:, None])  with rows=(j+i)%N, cols=j
    out = x @ W.T                                          # (256, 4096)

The scatter indices form a bijection (for each col j, row (j+i)%N hits every
row exactly once as i varies), so there is no actual accumulation:

    W[r, c] = V[(r - c) % N, c] * a[(r - c) % N]
    out[b, r] = sum_c x[b, c] * V[(r-c)%N, c] * a[(r-c)%N]

Sharding: output columns r are sharded 8 ways (512 per core) -> no collective;
each core reads only the diagonal band of V it needs, all of x, and produces a
disjoint out[:, r0:r0+512] slice.

The GEMM datapath runs in bf16 (tolerance is 2e-2; measured error ~4e-3):
the host ships the V band, x^T and alpha already in bf16 and in
partition-blocked layouts so every DMA is 128 large contiguous descriptors
(2-8 KB each).  Per-core HBM traffic is ~8 MB (band 4 MB + xT 2 MB +
Toeplitz scale windows 1.4 MB + out 0.25 MB) vs 16.8 MB for the fp32
version, and sustains 420+ GB/s.

Device-side layout trick: with the contraction rows presented in REVERSED
order (c = N-1-p for SBUF partition-row p), the skewed scale field the band
tiles need becomes the ascending Toeplitz  scale[p, j] = a2[1 + p + j]  where
a2 is `a` doubled.  Raw (doubled, rolled) alpha is DMA'd directly in this
overlapping-window layout (one DMA per batch, partition step +1); the
soft-topk transform is applied on-chip:

    a = min(exp(alpha) * K/sum(exp(alpha)), 1)

via one Exp pass per window on the otherwise-idle Scalar engine and a fused
mult+min tensor_scalar on Vector (bf16 single-src -> 4x mode), with 1/sum
from a ones-matmul partition broadcast + DVE reciprocal.  Only the Exp
activation table is loaded (less preamble); nothing round-trips through
DRAM.  (GpSimd's tensor_scalar ucode for this shape measured 17x slower
than DVE -- do not move the scale pass there.)

DMA scheduling (load-bearing, learned from traces):
  * ALL loads ride ONE HWDGE ring (sync) in strict batch order
    [alpha, (win_q, band_q, xt_q) x 5]: single-queue FIFO makes completion
    order == need order.  A second ring gets starved (SDMA round-robins
    rings per-DESCRIPTOR, so small-descriptor transfers lose ~4x to big
    ones), and the 8 shared completion-semaphore lanes recycle round-robin
    ACROSS rings, stalling later issues on whichever ring reuses a lane.
  * Keep 2-3 mid-size transfers (0.25-1 MB, 2-8 KB descriptors) in flight:
    one packed 1.9 MB transfer per batch (12-15 KB descriptors) measured
    ~280 GB/s vs 420+ for this mix.
  * Batch sizes [4, 8, 8, 8, 4]: small first batch starts the matmul
    pipeline early, small last batch keeps the post-DMA tail short.  The
    last batch loads xT before its band so the band is the only tail gate.
The two output halves take independent engine paths (DVE cast + scalar-ring
store / ACT copy + sync-ring store) so they fully overlap; tiny keep-alive
matmuls gated on each band arrival stop the PE HAM clock-gate from
re-throttling between matmul bursts.
"""

import os
import sys

import numpy as np

for _p in ("/opt/trn_rl_repo", "/root/.axon_site/_ro/trn_rl_repo"):
    if os.path.isdir(_p) and _p not in sys.path:
        sys.path.append(_p)

import ml_dtypes

import concourse.bacc as bacc
import concourse.bass as bass
import concourse.mybir as mybir
import concourse.tile as tile
from concourse.bass_utils import run_bass_kernel_spmd

F32 = mybir.dt.float32
BF16 = mybir.dt.bfloat16
NP_BF16 = ml_dtypes.bfloat16

N = 4096          # IN_F == OUT_F == N_PERM == DIAG
B = 256           # batch
NCORES = 8
RW = N // NCORES  # 512 output columns per core
K_TOPK = 3687     # ceil(0.9 * 4096 * 4096 / 4096)
CB = 128          # contraction block (SBUF partition count)
NCB = N // CB     # 32 contraction blocks
# contraction blocks per DMA/multiply batch: small first batch so the first
# matmuls start early, small last batch so the post-DMA tail is short.
# ([6,8,8,8,2] measured worse: the tiny last batch just shifts the serial
# tail onto batch 3's sixteen matmuls.)
BATCH_SIZES = [4, 8, 8, 8, 4]
BATCH_STARTS = [sum(BATCH_SIZES[:i]) for i in range(len(BATCH_SIZES))]
NBATCH = len(BATCH_SIZES)


def _strided_cols(ap2d, col_off, t_step, n_t, inner):
    """[128, W] SBUF tile -> [128, n_t, inner] view starting at col_off with
    column stride t_step between t-slices (overlap allowed)."""
    pstep = ap2d.ap[0][0]
    return bass.AP(
        ap2d.tensor, ap2d.offset + col_off,
        [[pstep, 128], [t_step, n_t], [1, inner]],
    )


def _build_program():
    nc = bacc.Bacc("TRN2", target_bir_lowering=False, debug=False)

    band = nc.dram_tensor("band", [128, NCB, RW], BF16, kind="ExternalInput").ap()
    xT = nc.dram_tensor("xT", [128, NCB, B], BF16, kind="ExternalInput").ap()
    alpha2 = nc.dram_tensor("alpha2", [2 * N], BF16, kind="ExternalInput").ap()
    out = nc.dram_tensor("out", [B, RW], BF16, kind="ExternalOutput").ap()

    with tile.TileContext(nc) as tc:
        with (
            tc.tile_pool(name="small", bufs=1) as sp,
            tc.tile_pool(name="graw", bufs=1) as grp,
            tc.tile_pool(name="gexp", bufs=3) as gxp,
            tc.tile_pool(name="gsc", bufs=3) as gwp,
            tc.tile_pool(name="vb", bufs=1) as vbp,
            tc.tile_pool(name="xtp", bufs=1) as xtp,
            tc.tile_pool(name="wt", bufs=4) as wtp,
            tc.tile_pool(name="opool", bufs=2) as op,
            tc.tile_pool(name="psum", bufs=1, space="PSUM") as pp,
            tc.tile_pool(name="psum_s", bufs=1, space="PSUM") as pps,
        ):
            # ---- input DMAs ----
            # ALL loads ride ONE HWDGE ring (sync) in strict batch order
            # [alpha, (w_q, band_q, xt_q) x 5]: single-queue FIFO makes
            # completion order == need order, and the mix of 2-8KB
            # descriptors across 2-3 in-flight transfers keeps the SDMA
            # engines fed (measured 420+ GB/s; a second ring gets starved
            # per-descriptor, and single huge packed transfers drop to
            # ~280 GB/s).
            alpha_sb = sp.tile([128, 2 * N // 128], BF16)
            nc.sync.dma_start(
                alpha_sb[:], alpha2[0 : 2 * N].rearrange("(p f) -> p f", p=128)
            )
            agr = [
                grp.tile([128, RW + (BATCH_SIZES[q] - 1) * CB], BF16,
                         name=f"agr{q}")
                for q in range(NBATCH)
            ]
            vb = [
                vbp.tile([128, BATCH_SIZES[q], RW], BF16, name=f"vb{q}")
                for q in range(NBATCH)
            ]
            xt = xtp.tile([128, NCB, B], BF16)

            def _dma_w(q):
                gpw = RW + (BATCH_SIZES[q] - 1) * CB
                src = bass.AP(
                    alpha2.tensor,
                    alpha2.offset + 1 + BATCH_STARTS[q] * CB,
                    [[1, 128], [1, gpw]],
                )
                nc.sync.dma_start(agr[q][:], src)

            def _dma_band(q):
                k0, tb = BATCH_STARTS[q], BATCH_SIZES[q]
                nc.sync.dma_start(vb[q][:], band[:, k0 : k0 + tb, :])

            def _dma_xt(q):
                k0, tb = BATCH_STARTS[q], BATCH_SIZES[q]
                nc.sync.dma_start(
                    xt[:, k0 : k0 + tb, :], xT[:, k0 : k0 + tb, :]
                )

            # steady batches stream [w_q, band_q, xt_q]; for the last batch
            # the xT chunk goes before the band, and the band arrives as two
            # half-DMAs, so the tail's TT/matmuls pipeline with the final
            # transfer instead of waiting for all of it
            for q in range(NBATCH - 1):
                _dma_w(q)
                _dma_band(q)
                _dma_xt(q)
            qL = NBATCH - 1
            k0L, tbL = BATCH_STARTS[qL], BATCH_SIZES[qL]
            hL = tbL // 2
            _dma_w(qL)
            _dma_xt(qL)
            nc.sync.dma_start(vb[qL][:, 0:hL, :], band[:, k0L : k0L + hL, :])
            nc.sync.dma_start(
                vb[qL][:, hL:tbL, :], band[:, k0L + hL : k0L + tbL, :]
            )

            def _vb(q):
                return vb[q][:]

            def _xt(q, t, b0, b1):
                return xt[:, BATCH_STARTS[q] + t, b0:b1]

            # ---- kinv = K / sum(exp(alpha)) broadcast to all partitions ----
            exp_sb = sp.tile([128, 2 * N // 128], F32)
            rowsum = sp.tile([128, 1], F32)
            # alpha is uniform in [0,1): no max-subtraction needed
            nc.scalar.activation(
                exp_sb[:], alpha_sb[:], mybir.ActivationFunctionType.Exp,
                accum_out=rowsum[:],
            )
            ones = sp.tile([128, 128], F32)
            nc.vector.memset(ones[:], 1.0)
            tot_ps = pps.tile([128, 1], F32)
            # total = ones.T @ rowsum -> per-partition copy of 2*sum
            nc.tensor.matmul(tot_ps[:], ones[:], rowsum[:], start=True, stop=True)
            inv = sp.tile([128, 1], F32)
            nc.vector.reciprocal(inv[:], tot_ps[:])
            kinv = sp.tile([128, 1], F32)
            # rowsum covered the doubled alpha -> tot = 2*sum, so scale by 2K
            nc.vector.tensor_scalar_mul(kinv[:], inv[:], 2.0 * K_TOPK)

            # ---- main loop ----
            psum0 = pp.tile([128, RW], F32)
            psum1 = pp.tile([128, RW], F32)
            psum_ka = pp.tile([128, 1], F32)
            for q in range(NBATCH):
                if q > 0:
                    # PE keep-alive: a tiny matmul gated on this batch's band
                    # arrival fires mid-gap between matmul bursts, so the
                    # HAM activity monitor never re-throttles the PE clock
                    # (cold matmuls run at 427-634ns instead of 216ns)
                    nc.tensor.matmul(
                        psum_ka[:], vb[q][:, 0, 0:128], vb[q][:, 0, 0:1],
                        start=True, stop=True,
                    )
                k0, tb = BATCH_STARTS[q], BATCH_SIZES[q]
                gpw = RW + (tb - 1) * CB
                # scale window: exp on Scalar (bf16 out), fused *kinv, min-1
                # on Vector (bf16 single-src -> 4x mode)
                agx = gxp.tile([128, gpw], BF16)
                nc.scalar.activation(
                    agx[:], agr[q][:], mybir.ActivationFunctionType.Exp
                )
                agw = gwp.tile([128, gpw], BF16)
                nc.vector.tensor_scalar(
                    agw[:], agx[:], kinv[:, 0:1], 1.0,
                    mybir.AluOpType.mult, mybir.AluOpType.min,
                )
                # scaled weights for this batch of tb contraction blocks
                wt = wtp.tile([128, tb, RW], BF16)
                if q < NBATCH - 1:
                    nc.vector.tensor_tensor(
                        wt[:], _vb(q), _strided_cols(agw, 0, CB, tb, RW),
                        mybir.AluOpType.mult,
                    )
                    for t in range(tb):
                        k = k0 + t
                        nc.tensor.matmul(psum0[:], _xt(q, t, 0, 128), wt[:, t, :],
                                         start=(k == 0), stop=False)
                        nc.tensor.matmul(psum1[:], _xt(q, t, 128, 256), wt[:, t, :],
                                         start=(k == 0), stop=False)
                else:
                    # tail: two half-TTs track the split band DMA; psum0's
                    # matmuls complete first so its cast+store overlaps
                    # psum1's remainder
                    h = tb // 2
                    nc.vector.tensor_tensor(
                        wt[:, 0:h, :], vb[q][:, 0:h, :],
                        _strided_cols(agw, 0, CB, h, RW),
                        mybir.AluOpType.mult,
                    )
                    nc.vector.tensor_tensor(
                        wt[:, h:tb, :], vb[q][:, h:tb, :],
                        _strided_cols(agw, h * CB, CB, tb - h, RW),
                        mybir.AluOpType.mult,
                    )
                    for t in range(h):
                        nc.tensor.matmul(psum0[:], _xt(q, t, 0, 128), wt[:, t, :],
                                         start=False, stop=False)
                        nc.tensor.matmul(psum1[:], _xt(q, t, 128, 256), wt[:, t, :],
                                         start=False, stop=False)
                    for t in range(h, tb):
                        k = k0 + t
                        nc.tensor.matmul(psum0[:], _xt(q, t, 0, 128), wt[:, t, :],
                                         start=False, stop=(k == NCB - 1))
                    for t in range(h, tb):
                        k = k0 + t
                        nc.tensor.matmul(psum1[:], _xt(q, t, 128, 256), wt[:, t, :],
                                         start=False, stop=(k == NCB - 1))

            # ---- PSUM -> SBUF -> DRAM (bf16 out; host widens to f32) ----
            # two independent engine paths so the halves fully overlap
            o0 = op.tile([128, RW], BF16)
            nc.vector.tensor_copy(o0[:], psum0[:])
            nc.scalar.dma_start(out[0:128, :], o0[:])
            o1 = op.tile([128, RW], BF16)
            nc.scalar.activation(
                o1[:], psum1[:], mybir.ActivationFunctionType.Copy
            )
            nc.sync.dma_start(out[128:256, :], o1[:])

    nc.compile()
    return nc


_NC_CACHE = []


def _get_program():
    if not _NC_CACHE:
        _NC_CACHE.append(_build_program())
    return _NC_CACHE[0]


def prepare_in_maps(x: np.ndarray, V: np.ndarray, alpha: np.ndarray):
    """Layout/dtype-only sharding of the full inputs into 8 per-core maps."""
    x = np.ascontiguousarray(np.asarray(x, dtype=np.float32))
    V = np.ascontiguousarray(np.asarray(V, dtype=np.float32))
    alpha = np.ascontiguousarray(np.asarray(alpha, dtype=np.float32))

    # rows presented in reversed order (c = N-1-p); see module docstring.
    # blocked [128, NCB, B] so each DMA chunk is contiguous per partition.
    xTb = np.ascontiguousarray(
        x.T[::-1, :].reshape(NCB, 128, B).transpose(1, 0, 2)
    ).astype(NP_BF16)

    # VtD[c, t] = V[t % N, c] for t in [0, 2N): doubled transpose for wrap-free
    # band extraction. band_m[c, j] = V[(r0 + j - c) % N, c]
    #              = VtD[c, N + r0 + j - c]
    Vt = np.ascontiguousarray(V.T)
    VtD = np.concatenate([Vt, Vt], axis=1)  # (N, 2N)
    flat = VtD.reshape(-1)
    isz = flat.itemsize

    in_maps = []
    for m in range(NCORES):
        r0 = m * RW
        start = N + r0  # element offset of band_m[0, 0] in flat
        band_m = np.lib.stride_tricks.as_strided(
            flat[start:], shape=(N, RW), strides=((2 * N - 1) * isz, isz),
        )
        band_b = np.ascontiguousarray(
            band_m[::-1, :].reshape(NCB, 128, RW).transpose(1, 0, 2)
        ).astype(NP_BF16)
        am = np.roll(alpha, -r0)
        in_maps.append({
            "band": band_b,
            "xT": xTb,
            "alpha2": np.concatenate([am, am]).astype(NP_BF16),
        })
    return in_maps


def gather_output(results) -> np.ndarray:
    return np.concatenate(
        [np.asarray(results[m]["out"], dtype=np.float32) for m in range(NCORES)],
        axis=1,
    )


def kernel(x: np.ndarray, V: np.ndarray, alpha: np.ndarray) -> np.ndarray:
    in_maps = prepare_in_maps(x, V, alpha)
    nc = _get_program()
    res = run_bass_kernel_spmd(nc, in_maps, core_ids=list(range(NCORES)))
    return gather_output(res.results)
